# revision 43
# baseline (speedup 1.0000x reference)
"""Causal multi-head attention layer for Trainium2 (Bass/Tile), 8 NeuronCores.

Problem: x[B=2,S=2048,D=1024], H=16 heads, Dh=64.
Sharding: data-parallel over batch (2) x tensor-parallel over head groups (4):
each of the 8 cores handles one batch element and 4 heads, producing a partial
output [S, D]; the host sums the 4 head-group partials per batch (the
"all-reduce after the W_O contraction" done host-side since we return full
output anyway) and adds biases that commute out (b_O and sum_h b_V[h] @ W_O[h],
exact because softmax rows sum to 1).

Device kernel (per core), all operands resident in SBUF:
  - x^T is fed pre-transposed from host: [128, KT=8, S] (D on partitions).
  - Q^T, K^T computed head-PAIR-packed: [128, NPAIR, S] (partitions 0:64 =
    head 2*pr dims, 64:128 = head 2*pr+1). W as stationary [128,128], x^T
    moving N=512.
  - V computed in [k, e] layout (x^T stationary, W_V moving N=256, all 4
    heads at once) and stored with an appended [1, 0] column pair: V'=[V|1|0].
  - Scores computed TRANSPOSED: S^T[k, q] = (K^T tile).T @ Q^T chunk, so
    softmax's sum lands on the matmul contraction instead of needing row
    reductions: Z'[e|1|0, q] = V'.T @ exp(S^T) accumulated over k-tiles gives
    both the unnormalized attention output (rows 0:64) and the softmax
    denominator l (row 64) in one accumulation. No max-subtraction is needed:
    scores are O(1) here, exp is safe in fp32.
  - Both heads of a pair write one 2-bank PSUM tile (disjoint PE row groups,
    so their K=64 matmuls run concurrently) and share a single 1024-wide
    ACTIVATE(Exp) to amortize the ~352-cycle ACT fixed cost.
  - Causal masking is multiplicative on exp(S^T), diagonal chunks only (on
    GpSimd, which is otherwise idle); fully-masked column ranges of diagonal
    chunks are skipped in the scores/exp/PV instructions.
  - The ones block of V' is replicated 64x, so l lands pre-broadcast on
    PV-accumulator partitions 64:128 and normalization is a wide DVE
    reciprocal_approx_fast + multiply — no cross-partition traffic. (The
    approx reciprocal must read the multi-matmul PSUM accumulation via an
    SBUF staging copy; reading PSUM directly returns garbage on HW.)
  - The kernel is PE-throughput-bound end to end (~91% Tensor busy in the
    flash region), so everything revolves around keeping the PE stream
    dense and dependency-free:
      * Phase 1 computes only the first two q-chunks' Q/K projections
        (8 PSUM groups fed ktile-by-ktile as the x^T DMA lands) and the
        first 8 V tiles; the rest of the Q/K and V projections ride inside
        the flash loop as deadline-ordered PE fill work (fill_queue), so
        the ACT exp stream starts ~25us earlier.
      * exp→PV runs at pipeline depth 2 (pends): the PV consuming exp(j)
        is emitted at j+2, so its ACT/GpSimd semaphores are long-satisfied
        and the PE never stalls on the hop (-11us vs depth 1).
      * The output projection (single K=128 matmuls per head pair — the
        pair-sum rides the contraction) is METERED (every 3rd j) through
        the middle chunks so a backlog of real PE work survives into the
        last chunk, whose own scores+PV underfill the ACT-paced loop; this
        replaced the old dummy filler matmuls and keeps the HAM clock-gate
        at 8/8 through the whole flash region without burning power budget.
      * Out-proj PSUM is evicted on DVE during flash (ACT paces the exp
        stream there) but on ACT during the final drain (ACT is idle then,
        DVE runs the normalize chains); the eviction casts to f16 so the
        out DMA traffic halves (host accumulates partials in f32).
  - Dummy warm-up matmuls run during the initial DMA load to ramp the PE
    p-state; input DMAs are interleaved ktile-by-ktile in first-use order
    (bqk first: it gates the first Q/K evictions and thus the flash start).
  - CAUTION: instruction *timings* here are extremely sensitive to SBUF
    tile layout. Innocuous-looking changes that shift pool allocations
    (adding a tile, growing a pool's bufs) have reproducibly slowed EVERY
    engine's instructions ~20% (SBUF port contention). Keep changes
    allocation-neutral or A/B against the previous layout.
"""

import os
import numpy as np

# 'f16'   = float16 operands: 2-byte moving operand streams at 1 PE
#           cycle/row (4-byte fp32/fp32r cost 2), 11-bit mantissa
# 'fp32r' = fp32 bits, single-pass reduced-precision PE mode (2 cyc/row)
# 'bf16'  = bf16 storage/matmuls (1 cyc/row, 8-bit mantissa)
# 'fp32'  = exact fp32 matmuls (two-pass, 4 cyc/row)
MM_MODE = os.environ.get("ATTN_MM_MODE", "f16")

P = 128
SC = 512  # q-chunk width (one PSUM bank of fp32)

_BUILD_CACHE = {}


def _np_sb(mm_mode):
    if mm_mode == "bf16":
        import ml_dtypes

        return np.dtype(ml_dtypes.bfloat16)
    if mm_mode == "f16":
        return np.dtype(np.float16)
    return np.dtype(np.float32)


def build_nc(S, Dm, NH, Dh, mm_mode, stage=99):
    """Build (and cache) the per-core Bass module. NH = heads per core."""
    key = (S, Dm, NH, Dh, mm_mode, stage)
    if key in _BUILD_CACHE:
        return _BUILD_CACHE[key]

    import concourse.bacc as bacc
    import concourse.mybir as mybir
    import concourse.tile as tile

    f32 = mybir.dt.float32
    # dt_w: dtype of every matmul operand. float32r data is fp32 bits that the
    # PE consumes in a single-pass reduced-precision mode; the BIR verifier
    # requires every fp32r matmul operand to be *produced* with float32r dtype
    # (DMA pass-through from a float32r DRAM tensor, or a compute-engine
    # write; memset cannot produce it).
    dt_w = {
        "bf16": mybir.dt.bfloat16,
        "f16": mybir.dt.float16,
        "fp32": mybir.dt.float32,
        "fp32r": mybir.dt.float32r,
    }[mm_mode]
    # dtype for non-matmul elementwise tiles (masks)
    dt_m = {
        "bf16": mybir.dt.bfloat16,
        "f16": mybir.dt.float16,
    }.get(mm_mode, mybir.dt.float32)

    KT = Dm // P       # k-tiles over the model dim (contraction of projections)
    NPAIR = NH // 2    # head pairs
    QC = S // SC       # q chunks
    NKT = S // P       # k-position tiles
    DH2 = Dm // SC     # output free-dim chunks
    assert Dh == 64 and NH % 2 == 0 and S % SC == 0 and Dm % SC == 0

    nc = bacc.Bacc(
        "TRN2",
        debug=False,
        enable_asserts=False,
        target_bir_lowering=False,
        num_devices=1,
    )

    xT_d = nc.dram_tensor("xT", [P, KT, S], dt_w, kind="ExternalInput")
    wqk_d = nc.dram_tensor("wqk", [P, KT, 2, NPAIR, P], dt_w, kind="ExternalInput")
    wv_d = nc.dram_tensor("wv", [P, KT, NH * Dh], dt_w, kind="ExternalInput")
    wo_d = nc.dram_tensor("wo", [P, NPAIR, Dm], dt_w, kind="ExternalInput")
    bqk_d = nc.dram_tensor("bqk", [P, 2, NPAIR], f32, kind="ExternalInput")
    # output in the 2-byte matmul dtype (halves the output DMA traffic; the
    # host accumulates head-group partials in f32, so only one rounding)
    dt_out = dt_w if mybir.dt.size(dt_w) == 2 else f32
    out_d = nc.dram_tensor("out", [S, Dm], dt_out, kind="ExternalOutput")

    def mm(ap):
        return ap

    Exp = mybir.ActivationFunctionType.Exp
    inv_sqrt_dh = 1.0 / float(np.sqrt(Dh))

    with tile.TileContext(nc) as tc:
        with tc.tile_pool(name="const", bufs=1) as cpool:
            # ---------- constants (DMAs emitted inside phase 1, ordered by
            # first use, so the PE starts after ~2 ktiles instead of the
            # whole 16MB input load) ----------
            wv = cpool.tile([P, KT, NH * Dh], dt_w)
            wo = cpool.tile([P, NPAIR, Dm], dt_w)
            bqk = cpool.tile([P, 2, NPAIR], f32)

            QTt = cpool.tile([P, NPAIR, S], dt_w)
            KTt = cpool.tile([P, NPAIR, S], dt_w)
            # V' = [V | 1...1]: the ones block is REPLICATED 64x so the PV
            # matmul broadcasts the softmax denominator l across output
            # partitions 64:128 (M=128 costs the same N cycles as M=65, and
            # 64-partition l lets the reciprocal run wide on DVE).
            Vt = cpool.tile([P, NKT, NH, 2 * Dh], dt_w)

            # causal masks for the 4 diagonal-chunk variants: keep (1.0) where
            # q >= k + v*128, else 0.0 (S^T layout: partition=k, free=q).
            # Built first, in a never-reused pool, so the GpSimd work (and its
            # library load) happens during the initial DMA wait.
            masks = cpool.tile([P, SC // P, SC], dt_m)
            nc.gpsimd.memset(masks[:], 1.0)
            for v in range(SC // P):
                nc.gpsimd.affine_select(
                    out=masks[:, v, :],
                    in_=masks[:, v, :],
                    compare_op=mybir.AluOpType.is_ge,
                    fill=0.0,
                    base=-(v * P),
                    pattern=[[1, SC]],
                    channel_multiplier=-1,
                )

            # ---------- phase 1: projections for the first two q-chunks
            # only; chunks 2..QC-1 are deferred into the flash loop as PE
            # fill work (so the ACT exp stream starts ~25us earlier) ----------
            with (
                tc.tile_pool(name="p1", bufs=1) as p1pool,
                tc.tile_pool(name="ps1", bufs=8, space="PSUM") as ps1,
            ):
                wqk = cpool.tile([P, KT, 2, NPAIR, P], dt_w)  # outlives
                # phase 1: deferred QK-projection steps read it in-flash
                xT = cpool.tile([P, KT, S], dt_w)  # outlives phase 1: the
                # deferred V-projection groups read it inside the flash loop
                # xT/wqk stream ktile-by-ktile on the Sync DMA queue (the
                # phase-1 QK matmuls are paced by these arrivals); the other
                # tensors issue in parallel from the otherwise-idle Scalar
                # queue (also HWDGE) so they neither consume xT issue slots
                # nor arrive late — the V groups were stalling ~1.3us on a
                # wv that sat behind four xT ktiles in the single queue
                nc.scalar.dma_start(bqk[:], bqk_d[:])
                nc.scalar.dma_start(wv[:], wv_d[:])
                nc.scalar.dma_start(wo[:], wo_d[:])
                for kt in range(KT):
                    nc.sync.dma_start(wqk[:, kt], wqk_d[:, kt])
                    nc.sync.dma_start(xT[:, kt, :], xT_d[:, kt, :])

                # HAM warm-up: dummy matmuls during the initial DMA wait so
                # the PE clock-gate is at 8/8 when real work arrives
                wst = p1pool.tile([P, SC], f32)
                nc.vector.memset(wst[:], 1.0)
                # preload the Exp table on the Scalar engine now (it's idle);
                # otherwise the first flash exp pays the ~1.3us table load
                # on the critical path
                tpre = p1pool.tile([1, 2], f32)
                nc.scalar.activation(tpre[:], wst[0:1, 0:2], Exp)
                wrm = p1pool.tile([P, SC], dt_w)
                nc.vector.tensor_copy(wrm[:], wst[:])
                nwu = 6 if dt_w is mybir.dt.float32r else 10
                pwu = ps1.tile([P, SC], f32, tag="mm")
                for i in range(nwu):
                    nc.tensor.matmul(
                        pwu[:], mm(wrm[:, 0:P]), mm(wrm[:]),
                        start=(i == 0), stop=(i == nwu - 1),
                    )

                # Q/K projections (first two q-chunks only): accumulate 8
                # PSUM groups at once so each arriving x^T ktile feeds 8
                # matmuls — the first pass over ktiles is DMA-paced and would
                # otherwise leave the PE mostly idle (re-throttling the
                # clock-gate)
                for qg in range(0, min(2, QC), 2):
                    qcs = list(range(qg, min(qg + 2, QC)))
                    pss = {
                        (pr, pj, qc): ps1.tile(
                            [P, SC], f32, tag="mm", name=f"psqk_{pr}_{pj}_{qc}"
                        )
                        for pr in range(NPAIR)
                        for pj in range(2)
                        for qc in qcs
                    }
                    for kt in range(KT):
                        st, sp = kt == 0, kt == KT - 1
                        for pr in range(NPAIR):
                            for pj in range(2):
                                for qc in qcs:
                                    xs = xT[:, kt, qc * SC : (qc + 1) * SC]
                                    nc.tensor.matmul(
                                        pss[(pr, pj, qc)][:],
                                        mm(wqk[:, kt, pj, pr, :]), mm(xs),
                                        start=st, stop=sp,
                                    )
                    for pr in range(NPAIR):
                        for qc in qcs:
                            qs1 = slice(qc * SC, (qc + 1) * SC)
                            # evict via ACT (idle during phase 1; Identity
                            # shares the preloaded Exp table so no table
                            # reload) — keeps DVE free for the V-tile CASTs
                            # so the V groups' PSUM recycles sooner
                            nc.scalar.activation(
                                QTt[:, pr, qs1], pss[(pr, 0, qc)][:],
                                mybir.ActivationFunctionType.Identity,
                                bias=bqk[:, 0, pr : pr + 1],
                            )
                            nc.scalar.activation(
                                KTt[:, pr, qs1], pss[(pr, 1, qc)][:],
                                mybir.ActivationFunctionType.Identity,
                                bias=bqk[:, 1, pr : pr + 1],
                            )

                # only the V tiles the first two flash chunks touch; the rest
                # are deferred into the flash loop as PE fill work
                for qt in range(min(2 * (SC // P), NKT)):
                    psV = ps1.tile([P, NH * Dh], f32, tag="mm")
                    for kt in range(KT):
                        nc.tensor.matmul(
                            psV[:],
                            mm(xT[:, kt, qt * P : (qt + 1) * P]),
                            mm(wv[:, kt, :]),
                            start=(kt == 0), stop=(kt == KT - 1),
                        )
                    nc.vector.tensor_copy(
                        Vt[:, qt, :, 0:Dh],
                        psV[:].rearrange("p (h e) -> p h e", e=Dh),
                    )

                # memset can't write float32r: stage the V' ones in f32, copy
                # over with a free-dim broadcast (needed first by the PV
                # matmuls in phase 2)
                cstage = p1pool.tile([P, 1, 1, Dh], f32)
                nc.vector.memset(cstage[:], 1.0)
                nc.vector.tensor_copy(
                    Vt[:, :, :, Dh : 2 * Dh],
                    cstage[:].to_broadcast((P, NKT, NH, Dh)),
                )

            # ---------- phases 2+3 ----------
            with tc.tile_pool(name="zt", bufs=1) as ztpool:
                ZTt = ztpool.tile([P, NPAIR, S], dt_w)
                self_flash(
                    nc, tc, stage, mm, Exp, inv_sqrt_dh, mybir,
                    QTt, KTt, Vt, ZTt, wo, out_d, masks, xT, wv, wqk, bqk,
                    S, Dm, Dh, NPAIR, QC, SC, P, DH2, KT, NKT, dt_w, dt_m, f32,
                )

    nc.compile()
    _BUILD_CACHE[key] = nc
    return nc


def self_flash(
    nc, tc, stage, mm, Exp, inv_sqrt_dh, mybir,
    QTt, KTt, Vt, ZTt, wo, out_d, masks, xT, wv, wqk, bqk,
    S, Dm, Dh, NPAIR, QC, SC, P, DH2, KT, NKT, dt_w, dt_m, f32,
):
    # ---------- phases 2+3: flash attention (scores transposed) with the
    # output projection interleaved one q-chunk behind ----------
    out_dt = dt_w if mybir.dt.size(dt_w) == 2 else f32
    with (
        tc.tile_pool(name="e", bufs=4) as epool,
        tc.tile_pool(name="r", bufs=4) as rpool,
        tc.tile_pool(name="o", bufs=4) as opool,
        tc.tile_pool(name="pss", bufs=2, space="PSUM") as ps_s,
        tc.tile_pool(name="psz", bufs=4, space="PSUM") as psz,
    ):
        if stage <= 1:
            nc.sync.dma_start(out_d[0:P, :], QTt[:, 0, 0:Dm])

        drain = [False]  # final-drain mode: outproj evictions move DVE→ACT

        def normalize(pr, qc, zA, zB):
            """ZT[:, q] = Z'[0:64, q] * (1 / l[q]); l arrives pre-broadcast
            on partitions 64:128 of the PV accumulators. DVE-only.
            (reciprocal_approx_fast must not read multi-matmul PSUM
            accumulations directly — stage l through SBUF first.)"""
            qs = slice(qc * SC, (qc + 1) * SC)
            rb = rpool.tile([64, 2, SC], f32, tag="rb")
            ls = rpool.tile([64, 2, SC], f32, tag="ls")
            nc.vector.tensor_copy(ls[:, 0, :], zA[Dh : 2 * Dh, :])
            nc.vector.tensor_copy(ls[:, 1, :], zB[Dh : 2 * Dh, :])
            nc.vector.reciprocal_approx_fast(rb[:], ls[:])
            nc.vector.tensor_mul(ZTt[0:64, pr, qs], zA[0:Dh, :], rb[:, 0, :])
            nc.vector.tensor_mul(ZTt[64:128, pr, qs], zB[0:Dh, :], rb[:, 1, :])

        def outproj_steps(qc):
            """Closures for this q-chunk's output projection, injected one at
            a time between later j-iterations to keep PE density high.
            out[q, d] = sum_h Z_h[q, :] @ W_O[h]; each K=128 matmul sums a
            head pair inside the contraction."""
            def step(t, dh2):
                def emit():
                    po = psz.tile([P, SC], f32, tag="z")
                    ds = slice(dh2 * SC, (dh2 + 1) * SC)
                    zs = slice(t * P, (t + 1) * P)
                    for pr in range(NPAIR):
                        nc.tensor.matmul(
                            po[:], mm(ZTt[:, pr, zs]), mm(wo[:, pr, ds]),
                            start=(pr == 0), stop=(pr == NPAIR - 1),
                        )
                    ot = opool.tile([P, SC], out_dt, tag="o")
                    # evict via DVE during flash (GpSimd cannot read PSUM;
                    # keeping this off the Scalar engine frees the exp stream
                    # that paces flash), but via ACT during the final drain:
                    # ACT is idle there and the DVE FIFO must stay clear for
                    # the per-tile normalize slices that gate these very
                    # steps (alternating engines here measured WORSE).
                    # Casting to the 2-byte output dtype halves the out DMA.
                    if drain[0]:
                        nc.scalar.activation(
                            ot[:], po[:], mybir.ActivationFunctionType.Copy
                        )
                    else:
                        nc.vector.tensor_copy(ot[:], po[:])
                    nc.sync.dma_start(out_d[t * P : (t + 1) * P, ds], ot[:])
                return emit

            return [
                step(t, dh2)
                for t in range(qc * (SC // P), (qc + 1) * (SC // P))
                for dh2 in range(DH2)
            ]

        def v_step(qt):
            """One deferred V-projection group: pure PE fill work for the
            flash loop. Must run before the chunk that reads Vt[qt]
            (qt tiles 4k..4k+3 are consumed first by q-chunk k)."""
            def emit():
                psV = psz.tile([P, NH * Dh], f32, tag="z", name=f"psv_{qt}")
                for kt in range(KT):
                    nc.tensor.matmul(
                        psV[:],
                        mm(xT[:, kt, qt * P : (qt + 1) * P]),
                        mm(wv[:, kt, :]),
                        start=(kt == 0), stop=(kt == KT - 1),
                    )
                nc.vector.tensor_copy(
                    Vt[:, qt, :, 0:Dh],
                    psV[:].rearrange("p (h e) -> p h e", e=Dh),
                )
            return emit

        def qk_step(qc, pr, pj):
            """One deferred Q/K-projection group (phase-1 work pushed into
            the flash loop as PE fill). Must run before chunk qc starts."""
            def emit():
                ps = psz.tile([P, SC], f32, tag="z", name=f"psqk{qc}_{pr}_{pj}")
                qs = slice(qc * SC, (qc + 1) * SC)
                for kt in range(KT):
                    nc.tensor.matmul(
                        ps[:], mm(wqk[:, kt, pj, pr, :]), mm(xT[:, kt, qs]),
                        start=(kt == 0), stop=(kt == KT - 1),
                    )
                dst = QTt if pj == 0 else KTt
                nc.vector.tensor_scalar_add(
                    dst[:, pr, qs], ps[:], bqk[:, pj, pr : pr + 1]
                )
            return emit

        NH = Vt.shape[2]
        # deadline-ordered fill work: (need-by-chunk, emit). Chunks 0/1 are
        # mostly PE-bound (their exp streams are short), so this projection
        # work rides there while ACT warms up.
        fill_queue = []
        for qc2 in range(2, QC):
            for pr2 in range(NPAIR):
                for pj2 in range(2):
                    fill_queue.append((qc2, qk_step(qc2, pr2, pj2)))
            for qt in range(qc2 * (SC // P), (qc2 + 1) * (SC // P)):
                fill_queue.append((qc2, v_step(qt)))
        op_queue = []
        chunk_tail = None
        for qc in range(QC if stage >= 2 else 0):
            # deadline safety net: any fill this chunk depends on that the
            # in-loop pops didn't get to yet runs now, up front
            while fill_queue and fill_queue[0][0] <= qc:
                fill_queue.pop(0)[1]()
            for pr in range(NPAIR):
                hA, hB = 2 * pr, 2 * pr + 1
                zA = psz.tile([P, SC], f32, tag="z")
                zB = psz.tile([P, SC], f32, tag="z")
                jmax = (qc + 1) * (SC // P)
                pends = []  # exp→PV pipeline, depth 2: the PV consuming
                # exp(j) is emitted at iteration j+2, so its semaphores are
                # long-satisfied and the PE never stalls on the ACT/GpSimd hop

                def emit_pv(j, eAB, c0, jmax=jmax, zA=zA, zB=zB, hA=hA, hB=hB):
                    st, sp = j == 0, j == jmax - 1
                    cs = slice(c0, SC)
                    nc.tensor.matmul(
                        zA[:, cs], mm(Vt[:, j, hA, :]), mm(eAB[:, 0, cs]),
                        start=st, stop=sp,
                    )
                    nc.tensor.matmul(
                        zB[:, cs], mm(Vt[:, j, hB, :]), mm(eAB[:, 1, cs]),
                        start=st, stop=sp,
                    )

                for j in range(jmax):
                    v = j - (jmax - SC // P)
                    # causal: columns below the diagonal tile are fully
                    # masked; skip them (fp32r moving dims must stay >= 256;
                    # 2-byte dtypes can slice all the way down)
                    if dt_w is mybir.dt.float32r:
                        c0 = min(v * P, 2 * P) if v > 0 else 0
                    else:
                        c0 = v * P if v > 0 else 0
                    cs = slice(c0, SC)
                    qf = slice(qc * SC + c0, (qc + 1) * SC)
                    sAB = ps_s.tile([P, 2, SC], f32, tag="s")
                    ks = slice(j * P, (j + 1) * P)
                    nc.tensor.matmul(
                        sAB[:, 0, cs],
                        mm(KTt[0:64, pr, ks]), mm(QTt[0:64, pr, qf]),
                        start=True, stop=True,
                    )
                    nc.tensor.matmul(
                        sAB[:, 1, cs],
                        mm(KTt[64:128, pr, ks]), mm(QTt[64:128, pr, qf]),
                        start=True, stop=True,
                    )
                    eAB = epool.tile([P, 2, SC], dt_w, tag="e")
                    nc.scalar.activation(
                        eAB[:, :, cs], sAB[:, :, cs], Exp, scale=inv_sqrt_dh
                    )
                    if v >= 0:  # chunk contains the causal diagonal
                        mv = slice(c0, min((v + 1) * P, SC))
                        nc.gpsimd.tensor_mul(
                            eAB[:, 0, mv], eAB[:, 0, mv], masks[:, v, mv]
                        )
                        nc.gpsimd.tensor_mul(
                            eAB[:, 1, mv], eAB[:, 1, mv], masks[:, v, mv]
                        )
                    if stage >= 3:
                        pends.append((j, eAB, c0))
                        if len(pends) > 3:
                            emit_pv(*pends.pop(0))
                        if j == 0 and chunk_tail is not None:
                            # cross-chunk pipeline: the previous chunk's last
                            # PV + normalize go here, AFTER this chunk's first
                            # scores pair is queued, so the exp stream never
                            # stalls at a chunk boundary
                            chunk_tail()
                            chunk_tail = None
                        elif j >= 1 and fill_queue:
                            fill_queue.pop(0)[1]()
                        elif j >= 2 and op_queue and (
                            qc == QC - 1 or j % 3 == 0
                        ):
                            # meter the out-proj drip in middle chunks so a
                            # backlog of real PE work survives into the last
                            # chunk, whose own PE work (scores+PV) underfills
                            # the ACT-paced loop — deliberate fill, replacing
                            # the old dummy filler matmuls
                            op_queue.pop(0)()
                    else:
                        last_e = eAB
                if stage < 3:
                    if pr == 0 and qc == 0:
                        nc.sync.dma_start(out_d[0:P, 0:SC], last_e[:, 0, :])
                    continue

                def chunk_tail(pends=pends, pr=pr, qc=qc, zA=zA, zB=zB,
                               emit_pv=emit_pv):
                    for p in pends:
                        emit_pv(*p)
                    normalize(pr, qc, zA, zB)
                    return pr, qc, zA, zB

            if stage >= 5:
                op_queue.extend(outproj_steps(qc))
        drain[0] = True
        if chunk_tail is not None:
            # final drain, pipelined: slice the last chunk's normalize per
            # q-tile and interleave that tile's out-proj steps, so the PE
            # overlaps the DVE multiplies instead of waiting for the full
            # chunk-width normalize
            pends, pr, qc, zA, zB = (chunk_tail.__defaults__[:5])
            for p in pends:
                chunk_tail.__defaults__[5](*p)
            rb = rpool.tile([64, 2, SC], f32, tag="rb")
            ls = rpool.tile([64, 2, SC], f32, tag="ls")
            for ti in range(SC // P):
                # whole DVE chain sliced per q-tile: each tile's out-proj
                # matmuls overlap the next tile's copies/reciprocal
                cl = slice(ti * P, (ti + 1) * P)
                qsl = slice(qc * SC + ti * P, qc * SC + (ti + 1) * P)
                nc.vector.tensor_copy(ls[:, 0, cl], zA[Dh : 2 * Dh, cl])
                nc.vector.tensor_copy(ls[:, 1, cl], zB[Dh : 2 * Dh, cl])
                nc.vector.reciprocal_approx_fast(rb[:, :, cl], ls[:, :, cl])
                nc.vector.tensor_mul(ZTt[0:64, pr, qsl], zA[0:Dh, cl], rb[:, 0, cl])
                nc.vector.tensor_mul(ZTt[64:128, pr, qsl], zB[0:Dh, cl], rb[:, 1, cl])
                for _ in range(DH2):
                    if op_queue:
                        op_queue.pop(0)()
        for step in op_queue:
            step()
        if stage == 4:
            nc.sync.dma_start(out_d[0:P, :], ZTt[:, 0, 0:Dm])


def pack_inputs(x_b, W_Q, W_K, W_V, W_O, b_Q, b_K, hds, mm_mode):
    """Host-side packing of one core's shard into the kernel's layouts."""
    npdt = _np_sb(mm_mode)
    Dm, Dh = W_Q.shape[1], W_Q.shape[2]
    S = x_b.shape[0]
    NH = len(hds)
    NPAIR = NH // 2
    KT = Dm // P

    xT = np.ascontiguousarray(
        x_b.T.reshape(KT, P, S).transpose(1, 0, 2)
    ).astype(npdt)

    def pack_w_in(W):  # [H, Dm, Dh] -> [P, KT, NPAIR, 2*Dh]
        W4 = np.asarray(W)[hds]  # [NH, Dm, Dh]
        t = W4.reshape(NPAIR, 2, KT, P, Dh).transpose(3, 2, 0, 1, 4)
        return t.reshape(P, KT, NPAIR, 2 * Dh)

    wqk = np.ascontiguousarray(
        np.stack([pack_w_in(W_Q), pack_w_in(W_K)], axis=2)  # [P, KT, 2, NPAIR, 128]
    ).astype(npdt)

    WV4 = np.asarray(W_V)[hds]  # [NH, Dm, Dh]
    wv = np.ascontiguousarray(
        WV4.reshape(NH, KT, P, Dh).transpose(2, 1, 0, 3).reshape(P, KT, NH * Dh)
    ).astype(npdt)

    WO4 = np.asarray(W_O)[hds]  # [NH, Dh, Dm]
    wo = np.ascontiguousarray(
        WO4.reshape(NPAIR, 2, Dh, Dm).transpose(1, 2, 0, 3).reshape(P, NPAIR, Dm)
    ).astype(npdt)

    def pack_b(b):  # [H, Dh] -> [P, NPAIR]
        b4 = np.asarray(b)[hds]
        return b4.reshape(NPAIR, 2, Dh).transpose(1, 2, 0).reshape(P, NPAIR)

    bqk = np.ascontiguousarray(
        np.stack([pack_b(b_Q), pack_b(b_K)], axis=1)  # [P, 2, NPAIR]
    ).astype(np.float32)

    return {"xT": xT, "wqk": wqk, "wv": wv, "wo": wo, "bqk": bqk}


def kernel(x, W_Q, W_K, W_V, W_O, b_Q, b_K, b_V, b_O, _trace=False):
    from concourse.bass_utils import run_bass_kernel_spmd

    x = np.asarray(x, np.float32)
    B, S, Dm = x.shape
    H, _, Dh = W_Q.shape
    NCORES = 8
    GB = NCORES // B        # head groups per batch element
    NH = H // GB            # heads per core

    nc = build_nc(S, Dm, NH, Dh, MM_MODE)

    in_maps = []
    for c in range(NCORES):
        b, g = c // GB, c % GB
        hds = list(range(g * NH, (g + 1) * NH))
        in_maps.append(
            pack_inputs(x[b], W_Q, W_K, W_V, W_O, b_Q, b_K, hds, MM_MODE)
        )

    try:
        res = run_bass_kernel_spmd(
            nc, in_maps, core_ids=list(range(NCORES)), trace=_trace
        )
    except Exception:
        # transient device hiccups (e.g. a wedged core from a previous run)
        # usually clear on retry
        res = run_bass_kernel_spmd(
            nc, in_maps, core_ids=list(range(NCORES)), trace=_trace
        )

    out = np.zeros((B, S, Dm), np.float32)
    for c in range(NCORES):
        out[c // GB] += res.results[c]["out"]

    # biases that commute out of the device kernel (softmax rows sum to 1)
    corr = np.asarray(b_O, np.float32) + np.einsum(
        "he,hed->d",
        np.asarray(b_V, np.float32),
        np.asarray(W_O, np.float32),
    )
    out += corr[None, None, :]

    if _trace:
        kernel.last_results = res
    return out



# revision 44
# speedup vs baseline: 1.0025x; 1.0025x over previous
"""Causal multi-head attention layer for Trainium2 (Bass/Tile), 8 NeuronCores.

Problem: x[B=2,S=2048,D=1024], H=16 heads, Dh=64.
Sharding: data-parallel over batch (2) x tensor-parallel over head groups (4):
each of the 8 cores handles one batch element and 4 heads, producing a partial
output [S, D]; the host sums the 4 head-group partials per batch (the
"all-reduce after the W_O contraction" done host-side since we return full
output anyway) and adds biases that commute out (b_O and sum_h b_V[h] @ W_O[h],
exact because softmax rows sum to 1).

Device kernel (per core), all operands resident in SBUF:
  - x^T is fed pre-transposed from host: [128, KT=8, S] (D on partitions).
  - Q^T, K^T computed head-PAIR-packed: [128, NPAIR, S] (partitions 0:64 =
    head 2*pr dims, 64:128 = head 2*pr+1). W as stationary [128,128], x^T
    moving N=512.
  - V computed in [k, e] layout (x^T stationary, W_V moving N=256, all 4
    heads at once) and stored with an appended [1, 0] column pair: V'=[V|1|0].
  - Scores computed TRANSPOSED: S^T[k, q] = (K^T tile).T @ Q^T chunk, so
    softmax's sum lands on the matmul contraction instead of needing row
    reductions: Z'[e|1|0, q] = V'.T @ exp(S^T) accumulated over k-tiles gives
    both the unnormalized attention output (rows 0:64) and the softmax
    denominator l (row 64) in one accumulation. No max-subtraction is needed:
    scores are O(1) here, exp is safe in fp32.
  - Both heads of a pair write one 2-bank PSUM tile (disjoint PE row groups,
    so their K=64 matmuls run concurrently) and share a single 1024-wide
    ACTIVATE(Exp) to amortize the ~352-cycle ACT fixed cost.
  - Causal masking is multiplicative on exp(S^T), diagonal chunks only (on
    GpSimd, which is otherwise idle); fully-masked column ranges of diagonal
    chunks are skipped in the scores/exp/PV instructions.
  - The ones block of V' is replicated 64x, so l lands pre-broadcast on
    PV-accumulator partitions 64:128 and normalization is a wide DVE
    reciprocal_approx_fast + multiply — no cross-partition traffic. (The
    approx reciprocal must read the multi-matmul PSUM accumulation via an
    SBUF staging copy; reading PSUM directly returns garbage on HW.)
  - The kernel is PE-throughput-bound end to end (~91% Tensor busy in the
    flash region), so everything revolves around keeping the PE stream
    dense and dependency-free:
      * Phase 1 computes only the first two q-chunks' Q/K projections
        (8 PSUM groups fed ktile-by-ktile as the x^T DMA lands) and the
        first 8 V tiles; the rest of the Q/K and V projections ride inside
        the flash loop as deadline-ordered PE fill work (fill_queue), so
        the ACT exp stream starts ~25us earlier.
      * exp→PV runs at pipeline depth 2 (pends): the PV consuming exp(j)
        is emitted at j+2, so its ACT/GpSimd semaphores are long-satisfied
        and the PE never stalls on the hop (-11us vs depth 1).
      * The output projection (single K=128 matmuls per head pair — the
        pair-sum rides the contraction) is METERED (every 3rd j) through
        the middle chunks so a backlog of real PE work survives into the
        last chunk, whose own scores+PV underfill the ACT-paced loop; this
        replaced the old dummy filler matmuls and keeps the HAM clock-gate
        at 8/8 through the whole flash region without burning power budget.
      * Out-proj PSUM is evicted on DVE during flash (ACT paces the exp
        stream there) but on ACT during the final drain (ACT is idle then,
        DVE runs the normalize chains); the eviction casts to f16 so the
        out DMA traffic halves (host accumulates partials in f32).
  - Dummy warm-up matmuls run during the initial DMA load to ramp the PE
    p-state; input DMAs are interleaved ktile-by-ktile in first-use order
    (bqk first: it gates the first Q/K evictions and thus the flash start).
  - CAUTION: instruction *timings* here are extremely sensitive to SBUF
    tile layout. Innocuous-looking changes that shift pool allocations
    (adding a tile, growing a pool's bufs) have reproducibly slowed EVERY
    engine's instructions ~20% (SBUF port contention). Keep changes
    allocation-neutral or A/B against the previous layout.
"""

import os
import numpy as np

# 'f16'   = float16 operands: 2-byte moving operand streams at 1 PE
#           cycle/row (4-byte fp32/fp32r cost 2), 11-bit mantissa
# 'fp32r' = fp32 bits, single-pass reduced-precision PE mode (2 cyc/row)
# 'bf16'  = bf16 storage/matmuls (1 cyc/row, 8-bit mantissa)
# 'fp32'  = exact fp32 matmuls (two-pass, 4 cyc/row)
MM_MODE = os.environ.get("ATTN_MM_MODE", "f16")

P = 128
SC = 512  # q-chunk width (one PSUM bank of fp32)

_BUILD_CACHE = {}


def _np_sb(mm_mode):
    if mm_mode == "bf16":
        import ml_dtypes

        return np.dtype(ml_dtypes.bfloat16)
    if mm_mode == "f16":
        return np.dtype(np.float16)
    return np.dtype(np.float32)


def build_nc(S, Dm, NH, Dh, mm_mode, stage=99):
    """Build (and cache) the per-core Bass module. NH = heads per core."""
    key = (S, Dm, NH, Dh, mm_mode, stage)
    if key in _BUILD_CACHE:
        return _BUILD_CACHE[key]

    import concourse.bacc as bacc
    import concourse.mybir as mybir
    import concourse.tile as tile

    f32 = mybir.dt.float32
    # dt_w: dtype of every matmul operand. float32r data is fp32 bits that the
    # PE consumes in a single-pass reduced-precision mode; the BIR verifier
    # requires every fp32r matmul operand to be *produced* with float32r dtype
    # (DMA pass-through from a float32r DRAM tensor, or a compute-engine
    # write; memset cannot produce it).
    dt_w = {
        "bf16": mybir.dt.bfloat16,
        "f16": mybir.dt.float16,
        "fp32": mybir.dt.float32,
        "fp32r": mybir.dt.float32r,
    }[mm_mode]
    # dtype for non-matmul elementwise tiles (masks)
    dt_m = {
        "bf16": mybir.dt.bfloat16,
        "f16": mybir.dt.float16,
    }.get(mm_mode, mybir.dt.float32)

    KT = Dm // P       # k-tiles over the model dim (contraction of projections)
    NPAIR = NH // 2    # head pairs
    QC = S // SC       # q chunks
    NKT = S // P       # k-position tiles
    DH2 = Dm // SC     # output free-dim chunks
    assert Dh == 64 and NH % 2 == 0 and S % SC == 0 and Dm % SC == 0

    nc = bacc.Bacc(
        "TRN2",
        debug=False,
        enable_asserts=False,
        target_bir_lowering=False,
        num_devices=1,
    )

    xT_d = nc.dram_tensor("xT", [P, KT, S], dt_w, kind="ExternalInput")
    wqk_d = nc.dram_tensor("wqk", [P, KT, 2, NPAIR, P], dt_w, kind="ExternalInput")
    wv_d = nc.dram_tensor("wv", [P, KT, NH * Dh], dt_w, kind="ExternalInput")
    wo_d = nc.dram_tensor("wo", [P, NPAIR, Dm], dt_w, kind="ExternalInput")
    bqk_d = nc.dram_tensor("bqk", [P, 2, NPAIR], f32, kind="ExternalInput")
    # output in the 2-byte matmul dtype (halves the output DMA traffic; the
    # host accumulates head-group partials in f32, so only one rounding)
    dt_out = dt_w if mybir.dt.size(dt_w) == 2 else f32
    out_d = nc.dram_tensor("out", [S, Dm], dt_out, kind="ExternalOutput")

    def mm(ap):
        return ap

    Exp = mybir.ActivationFunctionType.Exp
    inv_sqrt_dh = 1.0 / float(np.sqrt(Dh))

    with tile.TileContext(nc) as tc:
        with tc.tile_pool(name="const", bufs=1) as cpool:
            # ---------- constants (DMAs emitted inside phase 1, ordered by
            # first use, so the PE starts after ~2 ktiles instead of the
            # whole 16MB input load) ----------
            wv = cpool.tile([P, KT, NH * Dh], dt_w)
            wo = cpool.tile([P, NPAIR, Dm], dt_w)
            bqk = cpool.tile([P, 2, NPAIR], f32)

            QTt = cpool.tile([P, NPAIR, S], dt_w)
            KTt = cpool.tile([P, NPAIR, S], dt_w)
            # V' = [V | 1...1]: the ones block is REPLICATED 64x so the PV
            # matmul broadcasts the softmax denominator l across output
            # partitions 64:128 (M=128 costs the same N cycles as M=65, and
            # 64-partition l lets the reciprocal run wide on DVE).
            Vt = cpool.tile([P, NKT, NH, 2 * Dh], dt_w)

            # causal masks for the 4 diagonal-chunk variants: keep (1.0) where
            # q >= k + v*128, else 0.0 (S^T layout: partition=k, free=q).
            # Built first, in a never-reused pool, so the GpSimd work (and its
            # library load) happens during the initial DMA wait.
            masks = cpool.tile([P, SC // P, SC], dt_m)
            nc.gpsimd.memset(masks[:], 1.0)
            for v in range(SC // P):
                nc.gpsimd.affine_select(
                    out=masks[:, v, :],
                    in_=masks[:, v, :],
                    compare_op=mybir.AluOpType.is_ge,
                    fill=0.0,
                    base=-(v * P),
                    pattern=[[1, SC]],
                    channel_multiplier=-1,
                )

            # ---------- phase 1: projections for the first two q-chunks
            # only; chunks 2..QC-1 are deferred into the flash loop as PE
            # fill work (so the ACT exp stream starts ~25us earlier) ----------
            with (
                tc.tile_pool(name="p1", bufs=1) as p1pool,
                tc.tile_pool(name="ps1", bufs=8, space="PSUM") as ps1,
            ):
                wqk = cpool.tile([P, KT, 2, NPAIR, P], dt_w)  # outlives
                # phase 1: deferred QK-projection steps read it in-flash
                xT = cpool.tile([P, KT, S], dt_w)  # outlives phase 1: the
                # deferred V-projection groups read it inside the flash loop
                # interleave weight/activation loads ktile-by-ktile; slot the
                # later-needed tensors between them. bqk goes first (tiny,
                # and the first Q/K bias-add evictions gate the flash start).
                # NOTE: issuing wv/wo early on the parallel Scalar HWDGE
                # queue measured WORSE — they steal HBM bandwidth from the
                # xT stream exactly while the QK matmuls are arrival-paced.
                nc.sync.dma_start(bqk[:], bqk_d[:])
                for kt in range(KT):
                    nc.sync.dma_start(wqk[:, kt], wqk_d[:, kt])
                    nc.sync.dma_start(xT[:, kt, :], xT_d[:, kt, :])
                    if kt == KT // 2:
                        nc.sync.dma_start(wv[:], wv_d[:])
                nc.sync.dma_start(wo[:], wo_d[:])

                # HAM warm-up: dummy matmuls during the initial DMA wait so
                # the PE clock-gate is at 8/8 when real work arrives
                wst = p1pool.tile([P, SC], f32)
                nc.vector.memset(wst[:], 1.0)
                # preload the Exp table on the Scalar engine now (it's idle);
                # otherwise the first flash exp pays the ~1.3us table load
                # on the critical path
                tpre = p1pool.tile([1, 2], f32)
                nc.scalar.activation(tpre[:], wst[0:1, 0:2], Exp)
                wrm = p1pool.tile([P, SC], dt_w)
                nc.vector.tensor_copy(wrm[:], wst[:])
                nwu = 6 if dt_w is mybir.dt.float32r else 10
                pwu = ps1.tile([P, SC], f32, tag="mm")
                for i in range(nwu):
                    nc.tensor.matmul(
                        pwu[:], mm(wrm[:, 0:P]), mm(wrm[:]),
                        start=(i == 0), stop=(i == nwu - 1),
                    )

                # Q/K projections (first two q-chunks only): accumulate 8
                # PSUM groups at once so each arriving x^T ktile feeds 8
                # matmuls — the first pass over ktiles is DMA-paced and would
                # otherwise leave the PE mostly idle (re-throttling the
                # clock-gate)
                for qg in range(0, min(2, QC), 2):
                    qcs = list(range(qg, min(qg + 2, QC)))
                    pss = {
                        (pr, pj, qc): ps1.tile(
                            [P, SC], f32, tag="mm", name=f"psqk_{pr}_{pj}_{qc}"
                        )
                        for pr in range(NPAIR)
                        for pj in range(2)
                        for qc in qcs
                    }
                    for kt in range(KT):
                        st, sp = kt == 0, kt == KT - 1
                        for pr in range(NPAIR):
                            for pj in range(2):
                                for qc in qcs:
                                    xs = xT[:, kt, qc * SC : (qc + 1) * SC]
                                    nc.tensor.matmul(
                                        pss[(pr, pj, qc)][:],
                                        mm(wqk[:, kt, pj, pr, :]), mm(xs),
                                        start=st, stop=sp,
                                    )
                    for pr in range(NPAIR):
                        for qc in qcs:
                            qs1 = slice(qc * SC, (qc + 1) * SC)
                            # evict via ACT (idle during phase 1; Identity
                            # shares the preloaded Exp table so no table
                            # reload) — keeps DVE free for the V-tile CASTs
                            # so the V groups' PSUM recycles sooner
                            nc.scalar.activation(
                                QTt[:, pr, qs1], pss[(pr, 0, qc)][:],
                                mybir.ActivationFunctionType.Identity,
                                bias=bqk[:, 0, pr : pr + 1],
                            )
                            nc.scalar.activation(
                                KTt[:, pr, qs1], pss[(pr, 1, qc)][:],
                                mybir.ActivationFunctionType.Identity,
                                bias=bqk[:, 1, pr : pr + 1],
                            )

                # only the V tiles the first two flash chunks touch; the rest
                # are deferred into the flash loop as PE fill work
                for qt in range(min(2 * (SC // P), NKT)):
                    psV = ps1.tile([P, NH * Dh], f32, tag="mm")
                    for kt in range(KT):
                        nc.tensor.matmul(
                            psV[:],
                            mm(xT[:, kt, qt * P : (qt + 1) * P]),
                            mm(wv[:, kt, :]),
                            start=(kt == 0), stop=(kt == KT - 1),
                        )
                    nc.vector.tensor_copy(
                        Vt[:, qt, :, 0:Dh],
                        psV[:].rearrange("p (h e) -> p h e", e=Dh),
                    )

                # memset can't write float32r: stage the V' ones in f32, copy
                # over with a free-dim broadcast (needed first by the PV
                # matmuls in phase 2)
                cstage = p1pool.tile([P, 1, 1, Dh], f32)
                nc.vector.memset(cstage[:], 1.0)
                nc.vector.tensor_copy(
                    Vt[:, :, :, Dh : 2 * Dh],
                    cstage[:].to_broadcast((P, NKT, NH, Dh)),
                )

            # ---------- phases 2+3 ----------
            with tc.tile_pool(name="zt", bufs=1) as ztpool:
                ZTt = ztpool.tile([P, NPAIR, S], dt_w)
                self_flash(
                    nc, tc, stage, mm, Exp, inv_sqrt_dh, mybir,
                    QTt, KTt, Vt, ZTt, wo, out_d, masks, xT, wv, wqk, bqk,
                    S, Dm, Dh, NPAIR, QC, SC, P, DH2, KT, NKT, dt_w, dt_m, f32,
                )

    nc.compile()
    _BUILD_CACHE[key] = nc
    return nc


def self_flash(
    nc, tc, stage, mm, Exp, inv_sqrt_dh, mybir,
    QTt, KTt, Vt, ZTt, wo, out_d, masks, xT, wv, wqk, bqk,
    S, Dm, Dh, NPAIR, QC, SC, P, DH2, KT, NKT, dt_w, dt_m, f32,
):
    # ---------- phases 2+3: flash attention (scores transposed) with the
    # output projection interleaved one q-chunk behind ----------
    out_dt = dt_w if mybir.dt.size(dt_w) == 2 else f32
    with (
        tc.tile_pool(name="e", bufs=4) as epool,
        tc.tile_pool(name="r", bufs=4) as rpool,
        tc.tile_pool(name="o", bufs=4) as opool,
        tc.tile_pool(name="pss", bufs=2, space="PSUM") as ps_s,
        tc.tile_pool(name="psz", bufs=4, space="PSUM") as psz,
    ):
        if stage <= 1:
            nc.sync.dma_start(out_d[0:P, :], QTt[:, 0, 0:Dm])

        drain = [False]  # final-drain mode: outproj evictions move DVE→ACT

        def normalize(pr, qc, zA, zB):
            """ZT[:, q] = Z'[0:64, q] * (1 / l[q]); l arrives pre-broadcast
            on partitions 64:128 of the PV accumulators. DVE-only.
            (reciprocal_approx_fast must not read multi-matmul PSUM
            accumulations directly — stage l through SBUF first.)"""
            qs = slice(qc * SC, (qc + 1) * SC)
            rb = rpool.tile([64, 2, SC], f32, tag="rb")
            ls = rpool.tile([64, 2, SC], f32, tag="ls")
            nc.vector.tensor_copy(ls[:, 0, :], zA[Dh : 2 * Dh, :])
            nc.vector.tensor_copy(ls[:, 1, :], zB[Dh : 2 * Dh, :])
            nc.vector.reciprocal_approx_fast(rb[:], ls[:])
            nc.vector.tensor_mul(ZTt[0:64, pr, qs], zA[0:Dh, :], rb[:, 0, :])
            nc.vector.tensor_mul(ZTt[64:128, pr, qs], zB[0:Dh, :], rb[:, 1, :])

        def outproj_steps(qc):
            """Closures for this q-chunk's output projection, injected one at
            a time between later j-iterations to keep PE density high.
            out[q, d] = sum_h Z_h[q, :] @ W_O[h]; each K=128 matmul sums a
            head pair inside the contraction."""
            def step(t, dh2):
                def emit():
                    po = psz.tile([P, SC], f32, tag="z")
                    ds = slice(dh2 * SC, (dh2 + 1) * SC)
                    zs = slice(t * P, (t + 1) * P)
                    for pr in range(NPAIR):
                        nc.tensor.matmul(
                            po[:], mm(ZTt[:, pr, zs]), mm(wo[:, pr, ds]),
                            start=(pr == 0), stop=(pr == NPAIR - 1),
                        )
                    ot = opool.tile([P, SC], out_dt, tag="o")
                    # evict via DVE during flash (GpSimd cannot read PSUM;
                    # keeping this off the Scalar engine frees the exp stream
                    # that paces flash), but via ACT during the final drain:
                    # ACT is idle there and the DVE FIFO must stay clear for
                    # the per-tile normalize slices that gate these very
                    # steps (alternating engines here measured WORSE).
                    # Casting to the 2-byte output dtype halves the out DMA.
                    if drain[0]:
                        nc.scalar.activation(
                            ot[:], po[:], mybir.ActivationFunctionType.Copy
                        )
                    else:
                        nc.vector.tensor_copy(ot[:], po[:])
                    nc.sync.dma_start(out_d[t * P : (t + 1) * P, ds], ot[:])
                return emit

            return [
                step(t, dh2)
                for t in range(qc * (SC // P), (qc + 1) * (SC // P))
                for dh2 in range(DH2)
            ]

        def v_step(qt):
            """One deferred V-projection group: pure PE fill work for the
            flash loop. Must run before the chunk that reads Vt[qt]
            (qt tiles 4k..4k+3 are consumed first by q-chunk k)."""
            def emit():
                psV = psz.tile([P, NH * Dh], f32, tag="z", name=f"psv_{qt}")
                for kt in range(KT):
                    nc.tensor.matmul(
                        psV[:],
                        mm(xT[:, kt, qt * P : (qt + 1) * P]),
                        mm(wv[:, kt, :]),
                        start=(kt == 0), stop=(kt == KT - 1),
                    )
                nc.vector.tensor_copy(
                    Vt[:, qt, :, 0:Dh],
                    psV[:].rearrange("p (h e) -> p h e", e=Dh),
                )
            return emit

        def qk_step(qc, pr, pj):
            """One deferred Q/K-projection group (phase-1 work pushed into
            the flash loop as PE fill). Must run before chunk qc starts."""
            def emit():
                ps = psz.tile([P, SC], f32, tag="z", name=f"psqk{qc}_{pr}_{pj}")
                qs = slice(qc * SC, (qc + 1) * SC)
                for kt in range(KT):
                    nc.tensor.matmul(
                        ps[:], mm(wqk[:, kt, pj, pr, :]), mm(xT[:, kt, qs]),
                        start=(kt == 0), stop=(kt == KT - 1),
                    )
                dst = QTt if pj == 0 else KTt
                nc.vector.tensor_scalar_add(
                    dst[:, pr, qs], ps[:], bqk[:, pj, pr : pr + 1]
                )
            return emit

        NH = Vt.shape[2]
        # deadline-ordered fill work: (need-by-chunk, emit). Chunks 0/1 are
        # mostly PE-bound (their exp streams are short), so this projection
        # work rides there while ACT warms up.
        fill_queue = []
        for qc2 in range(2, QC):
            for pr2 in range(NPAIR):
                for pj2 in range(2):
                    fill_queue.append((qc2, qk_step(qc2, pr2, pj2)))
            for qt in range(qc2 * (SC // P), (qc2 + 1) * (SC // P)):
                fill_queue.append((qc2, v_step(qt)))
        op_queue = []
        chunk_tail = None
        for qc in range(QC if stage >= 2 else 0):
            # deadline safety net: any fill this chunk depends on that the
            # in-loop pops didn't get to yet runs now, up front
            while fill_queue and fill_queue[0][0] <= qc:
                fill_queue.pop(0)[1]()
            for pr in range(NPAIR):
                hA, hB = 2 * pr, 2 * pr + 1
                zA = psz.tile([P, SC], f32, tag="z")
                zB = psz.tile([P, SC], f32, tag="z")
                jmax = (qc + 1) * (SC // P)
                pends = []  # exp→PV pipeline, depth 2: the PV consuming
                # exp(j) is emitted at iteration j+2, so its semaphores are
                # long-satisfied and the PE never stalls on the ACT/GpSimd hop

                def emit_pv(j, eAB, c0, jmax=jmax, zA=zA, zB=zB, hA=hA, hB=hB):
                    st, sp = j == 0, j == jmax - 1
                    cs = slice(c0, SC)
                    nc.tensor.matmul(
                        zA[:, cs], mm(Vt[:, j, hA, :]), mm(eAB[:, 0, cs]),
                        start=st, stop=sp,
                    )
                    nc.tensor.matmul(
                        zB[:, cs], mm(Vt[:, j, hB, :]), mm(eAB[:, 1, cs]),
                        start=st, stop=sp,
                    )

                for j in range(jmax):
                    v = j - (jmax - SC // P)
                    # causal: columns below the diagonal tile are fully
                    # masked; skip them (fp32r moving dims must stay >= 256;
                    # 2-byte dtypes can slice all the way down)
                    if dt_w is mybir.dt.float32r:
                        c0 = min(v * P, 2 * P) if v > 0 else 0
                    else:
                        c0 = v * P if v > 0 else 0
                    cs = slice(c0, SC)
                    qf = slice(qc * SC + c0, (qc + 1) * SC)
                    sAB = ps_s.tile([P, 2, SC], f32, tag="s")
                    ks = slice(j * P, (j + 1) * P)
                    nc.tensor.matmul(
                        sAB[:, 0, cs],
                        mm(KTt[0:64, pr, ks]), mm(QTt[0:64, pr, qf]),
                        start=True, stop=True,
                    )
                    nc.tensor.matmul(
                        sAB[:, 1, cs],
                        mm(KTt[64:128, pr, ks]), mm(QTt[64:128, pr, qf]),
                        start=True, stop=True,
                    )
                    eAB = epool.tile([P, 2, SC], dt_w, tag="e")
                    nc.scalar.activation(
                        eAB[:, :, cs], sAB[:, :, cs], Exp, scale=inv_sqrt_dh
                    )
                    if v >= 0:  # chunk contains the causal diagonal
                        mv = slice(c0, min((v + 1) * P, SC))
                        nc.gpsimd.tensor_mul(
                            eAB[:, 0, mv], eAB[:, 0, mv], masks[:, v, mv]
                        )
                        nc.gpsimd.tensor_mul(
                            eAB[:, 1, mv], eAB[:, 1, mv], masks[:, v, mv]
                        )
                    if stage >= 3:
                        pends.append((j, eAB, c0))
                        if len(pends) > 3:
                            emit_pv(*pends.pop(0))
                        if j == 0 and chunk_tail is not None:
                            # cross-chunk pipeline: the previous chunk's last
                            # PV + normalize go here, AFTER this chunk's first
                            # scores pair is queued, so the exp stream never
                            # stalls at a chunk boundary
                            chunk_tail()
                            chunk_tail = None
                        elif j >= 1 and fill_queue:
                            fill_queue.pop(0)[1]()
                        elif j >= 2 and op_queue and (
                            qc == QC - 1 or j % 3 == 0
                        ):
                            # meter the out-proj drip in middle chunks so a
                            # backlog of real PE work survives into the last
                            # chunk, whose own PE work (scores+PV) underfills
                            # the ACT-paced loop — deliberate fill, replacing
                            # the old dummy filler matmuls
                            op_queue.pop(0)()
                    else:
                        last_e = eAB
                if stage < 3:
                    if pr == 0 and qc == 0:
                        nc.sync.dma_start(out_d[0:P, 0:SC], last_e[:, 0, :])
                    continue

                def chunk_tail(pends=pends, pr=pr, qc=qc, zA=zA, zB=zB,
                               emit_pv=emit_pv):
                    for p in pends:
                        emit_pv(*p)
                    normalize(pr, qc, zA, zB)
                    return pr, qc, zA, zB

            if stage >= 5:
                op_queue.extend(outproj_steps(qc))
        drain[0] = True
        if chunk_tail is not None:
            # final drain, pipelined: slice the last chunk's normalize per
            # q-tile and interleave that tile's out-proj steps, so the PE
            # overlaps the DVE multiplies instead of waiting for the full
            # chunk-width normalize
            pends, pr, qc, zA, zB = (chunk_tail.__defaults__[:5])
            for p in pends:
                chunk_tail.__defaults__[5](*p)
            rb = rpool.tile([64, 2, SC], f32, tag="rb")
            ls = rpool.tile([64, 2, SC], f32, tag="ls")
            for ti in range(SC // P):
                # whole DVE chain sliced per q-tile: each tile's out-proj
                # matmuls overlap the next tile's copies/reciprocal
                cl = slice(ti * P, (ti + 1) * P)
                qsl = slice(qc * SC + ti * P, qc * SC + (ti + 1) * P)
                nc.vector.tensor_copy(ls[:, 0, cl], zA[Dh : 2 * Dh, cl])
                nc.vector.tensor_copy(ls[:, 1, cl], zB[Dh : 2 * Dh, cl])
                nc.vector.reciprocal_approx_fast(rb[:, :, cl], ls[:, :, cl])
                nc.vector.tensor_mul(ZTt[0:64, pr, qsl], zA[0:Dh, cl], rb[:, 0, cl])
                nc.vector.tensor_mul(ZTt[64:128, pr, qsl], zB[0:Dh, cl], rb[:, 1, cl])
                for _ in range(DH2):
                    if op_queue:
                        op_queue.pop(0)()
        for step in op_queue:
            step()
        if stage == 4:
            nc.sync.dma_start(out_d[0:P, :], ZTt[:, 0, 0:Dm])


def pack_inputs(x_b, W_Q, W_K, W_V, W_O, b_Q, b_K, hds, mm_mode):
    """Host-side packing of one core's shard into the kernel's layouts."""
    npdt = _np_sb(mm_mode)
    Dm, Dh = W_Q.shape[1], W_Q.shape[2]
    S = x_b.shape[0]
    NH = len(hds)
    NPAIR = NH // 2
    KT = Dm // P

    xT = np.ascontiguousarray(
        x_b.T.reshape(KT, P, S).transpose(1, 0, 2)
    ).astype(npdt)

    def pack_w_in(W):  # [H, Dm, Dh] -> [P, KT, NPAIR, 2*Dh]
        W4 = np.asarray(W)[hds]  # [NH, Dm, Dh]
        t = W4.reshape(NPAIR, 2, KT, P, Dh).transpose(3, 2, 0, 1, 4)
        return t.reshape(P, KT, NPAIR, 2 * Dh)

    wqk = np.ascontiguousarray(
        np.stack([pack_w_in(W_Q), pack_w_in(W_K)], axis=2)  # [P, KT, 2, NPAIR, 128]
    ).astype(npdt)

    WV4 = np.asarray(W_V)[hds]  # [NH, Dm, Dh]
    wv = np.ascontiguousarray(
        WV4.reshape(NH, KT, P, Dh).transpose(2, 1, 0, 3).reshape(P, KT, NH * Dh)
    ).astype(npdt)

    WO4 = np.asarray(W_O)[hds]  # [NH, Dh, Dm]
    wo = np.ascontiguousarray(
        WO4.reshape(NPAIR, 2, Dh, Dm).transpose(1, 2, 0, 3).reshape(P, NPAIR, Dm)
    ).astype(npdt)

    def pack_b(b):  # [H, Dh] -> [P, NPAIR]
        b4 = np.asarray(b)[hds]
        return b4.reshape(NPAIR, 2, Dh).transpose(1, 2, 0).reshape(P, NPAIR)

    bqk = np.ascontiguousarray(
        np.stack([pack_b(b_Q), pack_b(b_K)], axis=1)  # [P, 2, NPAIR]
    ).astype(np.float32)

    return {"xT": xT, "wqk": wqk, "wv": wv, "wo": wo, "bqk": bqk}


def kernel(x, W_Q, W_K, W_V, W_O, b_Q, b_K, b_V, b_O, _trace=False):
    from concourse.bass_utils import run_bass_kernel_spmd

    x = np.asarray(x, np.float32)
    B, S, Dm = x.shape
    H, _, Dh = W_Q.shape
    NCORES = 8
    GB = NCORES // B        # head groups per batch element
    NH = H // GB            # heads per core

    nc = build_nc(S, Dm, NH, Dh, MM_MODE)

    in_maps = []
    for c in range(NCORES):
        b, g = c // GB, c % GB
        hds = list(range(g * NH, (g + 1) * NH))
        in_maps.append(
            pack_inputs(x[b], W_Q, W_K, W_V, W_O, b_Q, b_K, hds, MM_MODE)
        )

    try:
        res = run_bass_kernel_spmd(
            nc, in_maps, core_ids=list(range(NCORES)), trace=_trace
        )
    except Exception:
        # transient device hiccups (e.g. a wedged core from a previous run)
        # usually clear on retry
        res = run_bass_kernel_spmd(
            nc, in_maps, core_ids=list(range(NCORES)), trace=_trace
        )

    out = np.zeros((B, S, Dm), np.float32)
    for c in range(NCORES):
        out[c // GB] += res.results[c]["out"]

    # biases that commute out of the device kernel (softmax rows sum to 1)
    corr = np.asarray(b_O, np.float32) + np.einsum(
        "he,hed->d",
        np.asarray(b_V, np.float32),
        np.asarray(W_O, np.float32),
    )
    out += corr[None, None, :]

    if _trace:
        kernel.last_results = res
    return out



# revision 45
# speedup vs baseline: 1.0115x; 1.0090x over previous
"""Causal multi-head attention layer for Trainium2 (Bass/Tile), 8 NeuronCores.

Problem: x[B=2,S=2048,D=1024], H=16 heads, Dh=64.
Sharding: data-parallel over batch (2) x tensor-parallel over head groups (4):
each of the 8 cores handles one batch element and 4 heads, producing a partial
output [S, D]; the host sums the 4 head-group partials per batch (the
"all-reduce after the W_O contraction" done host-side since we return full
output anyway) and adds biases that commute out (b_O and sum_h b_V[h] @ W_O[h],
exact because softmax rows sum to 1).

Device kernel (per core), all operands resident in SBUF:
  - x^T is fed pre-transposed from host: [128, KT=8, S] (D on partitions).
  - Q^T, K^T computed head-PAIR-packed: [128, NPAIR, S] (partitions 0:64 =
    head 2*pr dims, 64:128 = head 2*pr+1). W as stationary [128,128], x^T
    moving N=512.
  - V computed in [k, e] layout (x^T stationary, W_V moving N=256, all 4
    heads at once) and stored with an appended [1, 0] column pair: V'=[V|1|0].
  - Scores computed TRANSPOSED: S^T[k, q] = (K^T tile).T @ Q^T chunk, so
    softmax's sum lands on the matmul contraction instead of needing row
    reductions: Z'[e|1|0, q] = V'.T @ exp(S^T) accumulated over k-tiles gives
    both the unnormalized attention output (rows 0:64) and the softmax
    denominator l (row 64) in one accumulation. No max-subtraction is needed:
    scores are O(1) here, exp is safe in fp32.
  - Both heads of a pair write one 2-bank PSUM tile (disjoint PE row groups,
    so their K=64 matmuls run concurrently) and share a single 1024-wide
    ACTIVATE(Exp) to amortize the ~352-cycle ACT fixed cost.
  - Causal masking is multiplicative on exp(S^T), diagonal chunks only (on
    GpSimd, which is otherwise idle); fully-masked column ranges of diagonal
    chunks are skipped in the scores/exp/PV instructions.
  - The ones block of V' is replicated 64x, so l lands pre-broadcast on
    PV-accumulator partitions 64:128 and normalization is a wide DVE
    reciprocal_approx_fast + multiply — no cross-partition traffic. (The
    approx reciprocal must read the multi-matmul PSUM accumulation via an
    SBUF staging copy; reading PSUM directly returns garbage on HW.)
  - The kernel is PE-throughput-bound end to end (~91% Tensor busy in the
    flash region), so everything revolves around keeping the PE stream
    dense and dependency-free:
      * Phase 1 computes only the first two q-chunks' Q/K projections
        (8 PSUM groups fed ktile-by-ktile as the x^T DMA lands) and the
        first 8 V tiles; the rest of the Q/K and V projections ride inside
        the flash loop as deadline-ordered PE fill work (fill_queue), so
        the ACT exp stream starts ~25us earlier.
      * exp→PV runs at pipeline depth 2 (pends): the PV consuming exp(j)
        is emitted at j+2, so its ACT/GpSimd semaphores are long-satisfied
        and the PE never stalls on the hop (-11us vs depth 1).
      * The output projection (single K=128 matmuls per head pair — the
        pair-sum rides the contraction) is METERED (every 3rd j) through
        the middle chunks so a backlog of real PE work survives into the
        last chunk, whose own scores+PV underfill the ACT-paced loop; this
        replaced the old dummy filler matmuls and keeps the HAM clock-gate
        at 8/8 through the whole flash region without burning power budget.
      * Out-proj PSUM is evicted on DVE during flash (ACT paces the exp
        stream there) but on ACT during the final drain (ACT is idle then,
        DVE runs the normalize chains); the eviction casts to f16 so the
        out DMA traffic halves (host accumulates partials in f32).
  - Dummy warm-up matmuls run during the initial DMA load to ramp the PE
    p-state; input DMAs are interleaved ktile-by-ktile in first-use order
    (bqk first: it gates the first Q/K evictions and thus the flash start).
  - CAUTION: instruction *timings* here are extremely sensitive to SBUF
    tile layout. Innocuous-looking changes that shift pool allocations
    (adding a tile, growing a pool's bufs) have reproducibly slowed EVERY
    engine's instructions ~20% (SBUF port contention). Keep changes
    allocation-neutral or A/B against the previous layout.
"""

import os
import numpy as np

# 'f16'   = float16 operands: 2-byte moving operand streams at 1 PE
#           cycle/row (4-byte fp32/fp32r cost 2), 11-bit mantissa
# 'fp32r' = fp32 bits, single-pass reduced-precision PE mode (2 cyc/row)
# 'bf16'  = bf16 storage/matmuls (1 cyc/row, 8-bit mantissa)
# 'fp32'  = exact fp32 matmuls (two-pass, 4 cyc/row)
MM_MODE = os.environ.get("ATTN_MM_MODE", "f16")

P = 128
SC = 512  # q-chunk width (one PSUM bank of fp32)

_BUILD_CACHE = {}


def _np_sb(mm_mode):
    if mm_mode == "bf16":
        import ml_dtypes

        return np.dtype(ml_dtypes.bfloat16)
    if mm_mode == "f16":
        return np.dtype(np.float16)
    return np.dtype(np.float32)


def build_nc(S, Dm, NH, Dh, mm_mode, stage=99):
    """Build (and cache) the per-core Bass module. NH = heads per core."""
    key = (S, Dm, NH, Dh, mm_mode, stage)
    if key in _BUILD_CACHE:
        return _BUILD_CACHE[key]

    import concourse.bacc as bacc
    import concourse.mybir as mybir
    import concourse.tile as tile

    f32 = mybir.dt.float32
    # dt_w: dtype of every matmul operand. float32r data is fp32 bits that the
    # PE consumes in a single-pass reduced-precision mode; the BIR verifier
    # requires every fp32r matmul operand to be *produced* with float32r dtype
    # (DMA pass-through from a float32r DRAM tensor, or a compute-engine
    # write; memset cannot produce it).
    dt_w = {
        "bf16": mybir.dt.bfloat16,
        "f16": mybir.dt.float16,
        "fp32": mybir.dt.float32,
        "fp32r": mybir.dt.float32r,
    }[mm_mode]
    # dtype for non-matmul elementwise tiles (masks)
    dt_m = {
        "bf16": mybir.dt.bfloat16,
        "f16": mybir.dt.float16,
    }.get(mm_mode, mybir.dt.float32)

    KT = Dm // P       # k-tiles over the model dim (contraction of projections)
    NPAIR = NH // 2    # head pairs
    QC = S // SC       # q chunks
    NKT = S // P       # k-position tiles
    DH2 = Dm // SC     # output free-dim chunks
    assert Dh == 64 and NH % 2 == 0 and S % SC == 0 and Dm % SC == 0

    nc = bacc.Bacc(
        "TRN2",
        debug=False,
        enable_asserts=False,
        target_bir_lowering=False,
        num_devices=1,
    )

    xT_d = nc.dram_tensor("xT", [P, KT, S], dt_w, kind="ExternalInput")
    wqk_d = nc.dram_tensor("wqk", [P, KT, 2, NPAIR, P], dt_w, kind="ExternalInput")
    wv_d = nc.dram_tensor("wv", [P, KT, NH * Dh], dt_w, kind="ExternalInput")
    wo_d = nc.dram_tensor("wo", [P, NPAIR, Dm], dt_w, kind="ExternalInput")
    bqk_d = nc.dram_tensor("bqk", [P, 2, NPAIR], f32, kind="ExternalInput")
    # output in the 2-byte matmul dtype (halves the output DMA traffic; the
    # host accumulates head-group partials in f32, so only one rounding)
    dt_out = dt_w if mybir.dt.size(dt_w) == 2 else f32
    out_d = nc.dram_tensor("out", [S, Dm], dt_out, kind="ExternalOutput")

    def mm(ap):
        return ap

    Exp = mybir.ActivationFunctionType.Exp
    inv_sqrt_dh = 1.0 / float(np.sqrt(Dh))

    with tile.TileContext(nc) as tc:
        with tc.tile_pool(name="const", bufs=1) as cpool:
            # ---------- constants (DMAs emitted inside phase 1, ordered by
            # first use, so the PE starts after ~2 ktiles instead of the
            # whole 16MB input load) ----------
            wv = cpool.tile([P, KT, NH * Dh], dt_w)
            wo = cpool.tile([P, NPAIR, Dm], dt_w)
            bqk = cpool.tile([P, 2, NPAIR], f32)

            QTt = cpool.tile([P, NPAIR, S], dt_w)
            KTt = cpool.tile([P, NPAIR, S], dt_w)
            # V' = [V | 1...1]: the ones block is REPLICATED 64x so the PV
            # matmul broadcasts the softmax denominator l across output
            # partitions 64:128 (M=128 costs the same N cycles as M=65, and
            # 64-partition l lets the reciprocal run wide on DVE).
            Vt = cpool.tile([P, NKT, NH, 2 * Dh], dt_w)

            # causal masks for the 4 diagonal-chunk variants: keep (1.0) where
            # q >= k + v*128, else 0.0 (S^T layout: partition=k, free=q).
            # Built first, in a never-reused pool, so the GpSimd work (and its
            # library load) happens during the initial DMA wait.
            masks = cpool.tile([P, SC // P, SC], dt_m)
            nc.gpsimd.memset(masks[:], 1.0)
            for v in range(SC // P):
                nc.gpsimd.affine_select(
                    out=masks[:, v, :],
                    in_=masks[:, v, :],
                    compare_op=mybir.AluOpType.is_ge,
                    fill=0.0,
                    base=-(v * P),
                    pattern=[[1, SC]],
                    channel_multiplier=-1,
                )

            # ---------- phase 1: projections for the first two q-chunks
            # only; chunks 2..QC-1 are deferred into the flash loop as PE
            # fill work (so the ACT exp stream starts ~25us earlier) ----------
            with (
                tc.tile_pool(name="p1", bufs=1) as p1pool,
                tc.tile_pool(name="ps1", bufs=8, space="PSUM") as ps1,
            ):
                wqk = cpool.tile([P, KT, 2, NPAIR, P], dt_w)  # outlives
                # phase 1: deferred QK-projection steps read it in-flash
                xT = cpool.tile([P, KT, S], dt_w)  # outlives phase 1: the
                # deferred V-projection groups read it inside the flash loop
                # interleave weight/activation loads ktile-by-ktile; slot the
                # later-needed tensors between them. bqk goes first (tiny,
                # and the first Q/K bias-add evictions gate the flash start).
                # NOTE: issuing wv/wo early on the parallel Scalar HWDGE
                # queue measured WORSE — they steal HBM bandwidth from the
                # xT stream exactly while the QK matmuls are arrival-paced.
                nc.sync.dma_start(bqk[:], bqk_d[:])
                for kt in range(KT):
                    nc.sync.dma_start(wqk[:, kt], wqk_d[:, kt])
                    nc.sync.dma_start(xT[:, kt, :], xT_d[:, kt, :])
                    if kt == KT // 2:
                        nc.sync.dma_start(wv[:], wv_d[:])
                nc.sync.dma_start(wo[:], wo_d[:])

                # HAM warm-up: dummy matmuls during the initial DMA wait so
                # the PE clock-gate is at 8/8 when real work arrives
                wst = p1pool.tile([P, SC], f32)
                nc.vector.memset(wst[:], 1.0)
                # preload the Exp table on the Scalar engine now (it's idle);
                # otherwise the first flash exp pays the ~1.3us table load
                # on the critical path
                tpre = p1pool.tile([1, 2], f32)
                nc.scalar.activation(tpre[:], wst[0:1, 0:2], Exp)
                wrm = p1pool.tile([P, SC], dt_w)
                nc.vector.tensor_copy(wrm[:], wst[:])
                nwu = 6 if dt_w is mybir.dt.float32r else 10
                pwu = ps1.tile([P, SC], f32, tag="mm")
                for i in range(nwu):
                    nc.tensor.matmul(
                        pwu[:], mm(wrm[:, 0:P]), mm(wrm[:]),
                        start=(i == 0), stop=(i == nwu - 1),
                    )

                # Q/K projections (first two q-chunks only): accumulate 8
                # PSUM groups at once so each arriving x^T ktile feeds 8
                # matmuls — the first pass over ktiles is DMA-paced and would
                # otherwise leave the PE mostly idle (re-throttling the
                # clock-gate)
                for qg in range(0, min(2, QC), 2):
                    qcs = list(range(qg, min(qg + 2, QC)))
                    pss = {
                        (pr, pj, qc): ps1.tile(
                            [P, SC], f32, tag="mm", name=f"psqk_{pr}_{pj}_{qc}"
                        )
                        for pr in range(NPAIR)
                        for pj in range(2)
                        for qc in qcs
                    }
                    for kt in range(KT):
                        st, sp = kt == 0, kt == KT - 1
                        for pr in range(NPAIR):
                            for pj in range(2):
                                for qc in qcs:
                                    xs = xT[:, kt, qc * SC : (qc + 1) * SC]
                                    nc.tensor.matmul(
                                        pss[(pr, pj, qc)][:],
                                        mm(wqk[:, kt, pj, pr, :]), mm(xs),
                                        start=st, stop=sp,
                                    )
                    for pr in range(NPAIR):
                        for qc in qcs:
                            qs1 = slice(qc * SC, (qc + 1) * SC)
                            # evict via ACT (idle during phase 1; Identity
                            # shares the preloaded Exp table so no table
                            # reload) — keeps DVE free for the V-tile CASTs
                            # so the V groups' PSUM recycles sooner
                            nc.scalar.activation(
                                QTt[:, pr, qs1], pss[(pr, 0, qc)][:],
                                mybir.ActivationFunctionType.Identity,
                                bias=bqk[:, 0, pr : pr + 1],
                            )
                            nc.scalar.activation(
                                KTt[:, pr, qs1], pss[(pr, 1, qc)][:],
                                mybir.ActivationFunctionType.Identity,
                                bias=bqk[:, 1, pr : pr + 1],
                            )

                # only the V tiles the first two flash chunks touch; the rest
                # are deferred into the flash loop as PE fill work
                for qt in range(min(2 * (SC // P), NKT)):
                    psV = ps1.tile([P, NH * Dh], f32, tag="mm")
                    for kt in range(KT):
                        nc.tensor.matmul(
                            psV[:],
                            mm(xT[:, kt, qt * P : (qt + 1) * P]),
                            mm(wv[:, kt, :]),
                            start=(kt == 0), stop=(kt == KT - 1),
                        )
                    nc.vector.tensor_copy(
                        Vt[:, qt, :, 0:Dh],
                        psV[:].rearrange("p (h e) -> p h e", e=Dh),
                    )

                # memset can't write float32r: stage the V' ones in f32, copy
                # over with a free-dim broadcast (needed first by the PV
                # matmuls in phase 2)
                cstage = p1pool.tile([P, 1, 1, Dh], f32)
                nc.vector.memset(cstage[:], 1.0)
                nc.vector.tensor_copy(
                    Vt[:, :, :, Dh : 2 * Dh],
                    cstage[:].to_broadcast((P, NKT, NH, Dh)),
                )

            # ---------- phases 2+3 ----------
            with tc.tile_pool(name="zt", bufs=1) as ztpool:
                ZTt = ztpool.tile([P, NPAIR, S], dt_w)
                self_flash(
                    nc, tc, stage, mm, Exp, inv_sqrt_dh, mybir,
                    QTt, KTt, Vt, ZTt, wo, out_d, masks, xT, wv, wqk, bqk,
                    S, Dm, Dh, NPAIR, QC, SC, P, DH2, KT, NKT, dt_w, dt_m, f32,
                )

    nc.compile()
    _BUILD_CACHE[key] = nc
    return nc


def self_flash(
    nc, tc, stage, mm, Exp, inv_sqrt_dh, mybir,
    QTt, KTt, Vt, ZTt, wo, out_d, masks, xT, wv, wqk, bqk,
    S, Dm, Dh, NPAIR, QC, SC, P, DH2, KT, NKT, dt_w, dt_m, f32,
):
    # ---------- phases 2+3: flash attention (scores transposed) with the
    # output projection interleaved one q-chunk behind ----------
    out_dt = dt_w if mybir.dt.size(dt_w) == 2 else f32
    with (
        tc.tile_pool(name="e", bufs=4) as epool,
        tc.tile_pool(name="r", bufs=4) as rpool,
        tc.tile_pool(name="o", bufs=4) as opool,
        tc.tile_pool(name="pss", bufs=2, space="PSUM") as ps_s,
        tc.tile_pool(name="psz", bufs=4, space="PSUM") as psz,
    ):
        if stage <= 1:
            nc.sync.dma_start(out_d[0:P, :], QTt[:, 0, 0:Dm])

        drain = [False]  # final-drain mode: outproj evictions move DVE→ACT

        def normalize(pr, qc, zA, zB):
            """ZT[:, q] = Z'[0:64, q] * (1 / l[q]); l arrives pre-broadcast
            on partitions 64:128 of the PV accumulators. DVE-only.
            (reciprocal_approx_fast must not read multi-matmul PSUM
            accumulations directly — stage l through SBUF first.)"""
            qs = slice(qc * SC, (qc + 1) * SC)
            rb = rpool.tile([64, 2, SC], f32, tag="rb")
            ls = rpool.tile([64, 2, SC], f32, tag="ls")
            nc.vector.tensor_copy(ls[:, 0, :], zA[Dh : 2 * Dh, :])
            nc.vector.tensor_copy(ls[:, 1, :], zB[Dh : 2 * Dh, :])
            nc.vector.reciprocal_approx_fast(rb[:], ls[:])
            nc.vector.tensor_mul(ZTt[0:64, pr, qs], zA[0:Dh, :], rb[:, 0, :])
            nc.vector.tensor_mul(ZTt[64:128, pr, qs], zB[0:Dh, :], rb[:, 1, :])

        def outproj_steps(qc):
            """Closures for this q-chunk's output projection, injected one at
            a time between later j-iterations to keep PE density high.
            out[q, d] = sum_h Z_h[q, :] @ W_O[h]; each K=128 matmul sums a
            head pair inside the contraction."""
            def step(t, dh2):
                def emit():
                    po = psz.tile([P, SC], f32, tag="z")
                    ds = slice(dh2 * SC, (dh2 + 1) * SC)
                    zs = slice(t * P, (t + 1) * P)
                    for pr in range(NPAIR):
                        nc.tensor.matmul(
                            po[:], mm(ZTt[:, pr, zs]), mm(wo[:, pr, ds]),
                            start=(pr == 0), stop=(pr == NPAIR - 1),
                        )
                    ot = opool.tile([P, SC], out_dt, tag="o")
                    # evict via DVE during flash (GpSimd cannot read PSUM;
                    # keeping this off the Scalar engine frees the exp stream
                    # that paces flash), but via ACT during the final drain:
                    # ACT is idle there and the DVE FIFO must stay clear for
                    # the per-tile normalize slices that gate these very
                    # steps (alternating engines here measured WORSE).
                    # Casting to the 2-byte output dtype halves the out DMA.
                    if drain[0]:
                        nc.scalar.activation(
                            ot[:], po[:], mybir.ActivationFunctionType.Copy
                        )
                    else:
                        nc.vector.tensor_copy(ot[:], po[:])
                    nc.sync.dma_start(out_d[t * P : (t + 1) * P, ds], ot[:])
                return emit

            return [
                step(t, dh2)
                for t in range(qc * (SC // P), (qc + 1) * (SC // P))
                for dh2 in range(DH2)
            ]

        def v_step(qt):
            """One deferred V-projection group: pure PE fill work for the
            flash loop. Must run before the chunk that reads Vt[qt]
            (qt tiles 4k..4k+3 are consumed first by q-chunk k)."""
            def emit():
                psV = psz.tile([P, NH * Dh], f32, tag="z", name=f"psv_{qt}")
                for kt in range(KT):
                    nc.tensor.matmul(
                        psV[:],
                        mm(xT[:, kt, qt * P : (qt + 1) * P]),
                        mm(wv[:, kt, :]),
                        start=(kt == 0), stop=(kt == KT - 1),
                    )
                nc.vector.tensor_copy(
                    Vt[:, qt, :, 0:Dh],
                    psV[:].rearrange("p (h e) -> p h e", e=Dh),
                )
            return emit

        def qk_step(qc, pr, pj):
            """One deferred Q/K-projection group (phase-1 work pushed into
            the flash loop as PE fill). Must run before chunk qc starts."""
            def emit():
                ps = psz.tile([P, SC], f32, tag="z", name=f"psqk{qc}_{pr}_{pj}")
                qs = slice(qc * SC, (qc + 1) * SC)
                for kt in range(KT):
                    nc.tensor.matmul(
                        ps[:], mm(wqk[:, kt, pj, pr, :]), mm(xT[:, kt, qs]),
                        start=(kt == 0), stop=(kt == KT - 1),
                    )
                dst = QTt if pj == 0 else KTt
                nc.vector.tensor_scalar_add(
                    dst[:, pr, qs], ps[:], bqk[:, pj, pr : pr + 1]
                )
            return emit

        NH = Vt.shape[2]
        # deadline-ordered fill work: (need-by-chunk, emit). Chunks 0/1 are
        # mostly PE-bound (their exp streams are short), so this projection
        # work rides there while ACT warms up.
        fill_queue = []
        for qc2 in range(2, QC):
            for pr2 in range(NPAIR):
                for pj2 in range(2):
                    fill_queue.append((qc2, qk_step(qc2, pr2, pj2)))
            for qt in range(qc2 * (SC // P), (qc2 + 1) * (SC // P)):
                fill_queue.append((qc2, v_step(qt)))
        op_queue = []
        chunk_tail = None
        for qc in range(QC if stage >= 2 else 0):
            # deadline safety net: any fill this chunk depends on that the
            # in-loop pops didn't get to yet runs now, up front
            while fill_queue and fill_queue[0][0] <= qc:
                fill_queue.pop(0)[1]()
            for pr in range(NPAIR):
                hA, hB = 2 * pr, 2 * pr + 1
                zA = psz.tile([P, SC], f32, tag="z")
                zB = psz.tile([P, SC], f32, tag="z")
                jmax = (qc + 1) * (SC // P)
                pends = []  # exp→PV pipeline, depth 2: the PV consuming
                # exp(j) is emitted at iteration j+2, so its semaphores are
                # long-satisfied and the PE never stalls on the ACT/GpSimd hop

                def emit_pv(j, eAB, c0, jmax=jmax, zA=zA, zB=zB, hA=hA, hB=hB):
                    st, sp = j == 0, j == jmax - 1
                    cs = slice(c0, SC)
                    nc.tensor.matmul(
                        zA[:, cs], mm(Vt[:, j, hA, :]), mm(eAB[:, 0, cs]),
                        start=st, stop=sp,
                    )
                    nc.tensor.matmul(
                        zB[:, cs], mm(Vt[:, j, hB, :]), mm(eAB[:, 1, cs]),
                        start=st, stop=sp,
                    )

                for j in range(jmax):
                    v = j - (jmax - SC // P)
                    # causal: columns below the diagonal tile are fully
                    # masked; skip them (fp32r moving dims must stay >= 256;
                    # 2-byte dtypes can slice all the way down)
                    if dt_w is mybir.dt.float32r:
                        c0 = min(v * P, 2 * P) if v > 0 else 0
                    else:
                        c0 = v * P if v > 0 else 0
                    cs = slice(c0, SC)
                    qf = slice(qc * SC + c0, (qc + 1) * SC)
                    sAB = ps_s.tile([P, 2, SC], f32, tag="s")
                    ks = slice(j * P, (j + 1) * P)
                    nc.tensor.matmul(
                        sAB[:, 0, cs],
                        mm(KTt[0:64, pr, ks]), mm(QTt[0:64, pr, qf]),
                        start=True, stop=True,
                    )
                    nc.tensor.matmul(
                        sAB[:, 1, cs],
                        mm(KTt[64:128, pr, ks]), mm(QTt[64:128, pr, qf]),
                        start=True, stop=True,
                    )
                    eAB = epool.tile([P, 2, SC], dt_w, tag="e")
                    nc.scalar.activation(
                        eAB[:, :, cs], sAB[:, :, cs], Exp, scale=inv_sqrt_dh
                    )
                    if v >= 0:  # chunk contains the causal diagonal
                        mv = slice(c0, min((v + 1) * P, SC))
                        nc.gpsimd.tensor_mul(
                            eAB[:, 0, mv], eAB[:, 0, mv], masks[:, v, mv]
                        )
                        nc.gpsimd.tensor_mul(
                            eAB[:, 1, mv], eAB[:, 1, mv], masks[:, v, mv]
                        )
                    if stage >= 3:
                        pends.append((j, eAB, c0))
                        if len(pends) > 3:
                            emit_pv(*pends.pop(0))
                        if j == 0 and chunk_tail is not None:
                            # cross-chunk pipeline: the previous chunk's last
                            # PV + normalize go here, AFTER this chunk's first
                            # scores pair is queued, so the exp stream never
                            # stalls at a chunk boundary
                            chunk_tail()
                            chunk_tail = None
                        elif j >= 1 and fill_queue:
                            fill_queue.pop(0)[1]()
                        elif j >= 2 and op_queue and (
                            qc == QC - 1 or j % 3 == 0
                        ):
                            # meter the out-proj drip in middle chunks so a
                            # backlog of real PE work survives into the last
                            # chunk, whose own PE work (scores+PV) underfills
                            # the ACT-paced loop — deliberate fill, replacing
                            # the old dummy filler matmuls
                            op_queue.pop(0)()
                    else:
                        last_e = eAB
                if stage < 3:
                    if pr == 0 and qc == 0:
                        nc.sync.dma_start(out_d[0:P, 0:SC], last_e[:, 0, :])
                    continue

                # pr-boundary cover: the next pair's first scores matmul
                # waits ~1us for ACT to drain this pair's last exps (ps_s
                # buffer release); give the PE an independent out-proj step
                # here so it works through that window instead of stalling
                if op_queue and qc >= 1:
                    op_queue.pop(0)()

                def chunk_tail(pends=pends, pr=pr, qc=qc, zA=zA, zB=zB,
                               emit_pv=emit_pv):
                    for p in pends:
                        emit_pv(*p)
                    normalize(pr, qc, zA, zB)
                    return pr, qc, zA, zB

            if stage >= 5:
                op_queue.extend(outproj_steps(qc))
        drain[0] = True
        if chunk_tail is not None:
            # final drain, pipelined: slice the last chunk's normalize per
            # q-tile and interleave that tile's out-proj steps, so the PE
            # overlaps the DVE multiplies instead of waiting for the full
            # chunk-width normalize
            pends, pr, qc, zA, zB = (chunk_tail.__defaults__[:5])
            for p in pends:
                chunk_tail.__defaults__[5](*p)
            rb = rpool.tile([64, 2, SC], f32, tag="rb")
            ls = rpool.tile([64, 2, SC], f32, tag="ls")
            for ti in range(SC // P):
                # whole DVE chain sliced per q-tile: each tile's out-proj
                # matmuls overlap the next tile's copies/reciprocal
                cl = slice(ti * P, (ti + 1) * P)
                qsl = slice(qc * SC + ti * P, qc * SC + (ti + 1) * P)
                nc.vector.tensor_copy(ls[:, 0, cl], zA[Dh : 2 * Dh, cl])
                nc.vector.tensor_copy(ls[:, 1, cl], zB[Dh : 2 * Dh, cl])
                nc.vector.reciprocal_approx_fast(rb[:, :, cl], ls[:, :, cl])
                nc.vector.tensor_mul(ZTt[0:64, pr, qsl], zA[0:Dh, cl], rb[:, 0, cl])
                nc.vector.tensor_mul(ZTt[64:128, pr, qsl], zB[0:Dh, cl], rb[:, 1, cl])
                for _ in range(DH2):
                    if op_queue:
                        op_queue.pop(0)()
        for step in op_queue:
            step()
        if stage == 4:
            nc.sync.dma_start(out_d[0:P, :], ZTt[:, 0, 0:Dm])


def pack_inputs(x_b, W_Q, W_K, W_V, W_O, b_Q, b_K, hds, mm_mode):
    """Host-side packing of one core's shard into the kernel's layouts."""
    npdt = _np_sb(mm_mode)
    Dm, Dh = W_Q.shape[1], W_Q.shape[2]
    S = x_b.shape[0]
    NH = len(hds)
    NPAIR = NH // 2
    KT = Dm // P

    xT = np.ascontiguousarray(
        x_b.T.reshape(KT, P, S).transpose(1, 0, 2)
    ).astype(npdt)

    def pack_w_in(W):  # [H, Dm, Dh] -> [P, KT, NPAIR, 2*Dh]
        W4 = np.asarray(W)[hds]  # [NH, Dm, Dh]
        t = W4.reshape(NPAIR, 2, KT, P, Dh).transpose(3, 2, 0, 1, 4)
        return t.reshape(P, KT, NPAIR, 2 * Dh)

    wqk = np.ascontiguousarray(
        np.stack([pack_w_in(W_Q), pack_w_in(W_K)], axis=2)  # [P, KT, 2, NPAIR, 128]
    ).astype(npdt)

    WV4 = np.asarray(W_V)[hds]  # [NH, Dm, Dh]
    wv = np.ascontiguousarray(
        WV4.reshape(NH, KT, P, Dh).transpose(2, 1, 0, 3).reshape(P, KT, NH * Dh)
    ).astype(npdt)

    WO4 = np.asarray(W_O)[hds]  # [NH, Dh, Dm]
    wo = np.ascontiguousarray(
        WO4.reshape(NPAIR, 2, Dh, Dm).transpose(1, 2, 0, 3).reshape(P, NPAIR, Dm)
    ).astype(npdt)

    def pack_b(b):  # [H, Dh] -> [P, NPAIR]
        b4 = np.asarray(b)[hds]
        return b4.reshape(NPAIR, 2, Dh).transpose(1, 2, 0).reshape(P, NPAIR)

    bqk = np.ascontiguousarray(
        np.stack([pack_b(b_Q), pack_b(b_K)], axis=1)  # [P, 2, NPAIR]
    ).astype(np.float32)

    return {"xT": xT, "wqk": wqk, "wv": wv, "wo": wo, "bqk": bqk}


def kernel(x, W_Q, W_K, W_V, W_O, b_Q, b_K, b_V, b_O, _trace=False):
    from concourse.bass_utils import run_bass_kernel_spmd

    x = np.asarray(x, np.float32)
    B, S, Dm = x.shape
    H, _, Dh = W_Q.shape
    NCORES = 8
    GB = NCORES // B        # head groups per batch element
    NH = H // GB            # heads per core

    nc = build_nc(S, Dm, NH, Dh, MM_MODE)

    in_maps = []
    for c in range(NCORES):
        b, g = c // GB, c % GB
        hds = list(range(g * NH, (g + 1) * NH))
        in_maps.append(
            pack_inputs(x[b], W_Q, W_K, W_V, W_O, b_Q, b_K, hds, MM_MODE)
        )

    try:
        res = run_bass_kernel_spmd(
            nc, in_maps, core_ids=list(range(NCORES)), trace=_trace
        )
    except Exception:
        # transient device hiccups (e.g. a wedged core from a previous run)
        # usually clear on retry
        res = run_bass_kernel_spmd(
            nc, in_maps, core_ids=list(range(NCORES)), trace=_trace
        )

    out = np.zeros((B, S, Dm), np.float32)
    for c in range(NCORES):
        out[c // GB] += res.results[c]["out"]

    # biases that commute out of the device kernel (softmax rows sum to 1)
    corr = np.asarray(b_O, np.float32) + np.einsum(
        "he,hed->d",
        np.asarray(b_V, np.float32),
        np.asarray(W_O, np.float32),
    )
    out += corr[None, None, :]

    if _trace:
        kernel.last_results = res
    return out

